# revision 2
# baseline (speedup 1.0000x reference)
"""Expert-parallel MoE FFN kernel for Trainium2 (Bass/Tile).

Problem: y[b,e,n,:] = gelu(x[b,e,n,:] @ w1[e] + b1[e]) @ w2[e] + b2[e]
Shapes:  x (2,8,2048,1024), w1 (8,1024,4096), b1 (8,4096),
         w2 (8,4096,1024), b2 (8,1024)  -> out (2,8,2048,1024) fp32.

Sharding: expert-parallel, one expert per NeuronCore (8 cores).  Each core
processes its expert's 4096 tokens through the full FFN locally; no
cross-core communication.

Per-core dataflow (all matmuls in float32r at N=512 -> full PE rate):
  Phase 1:  xT = transpose(x_e)  (PE transpose, 128x128 tiles)
            hT[h,t] = gelu(sum_d w1[d,h] * xT[d,t] + b1[h])   (hT: [H,T])
            hT staged to DRAM (doesn't fit SBUF alongside weights).
  Phase 2:  y[t,d] = sum_h hT[h,t] * w2[h,d] (+ b2)
The hT (activation-transposed) layout means the big [H,T] intermediate is
produced and consumed with no transposes; only x needs a transpose on the
way in, and y comes out in natural [T,D] layout.
"""

import numpy as np
from contextlib import ExitStack

import concourse.bass as bass
import concourse.mybir as mybir
import concourse.tile as tile
from concourse import bacc
from concourse.bass_utils import run_bass_kernel_spmd
from concourse.masks import make_identity

P = 128
F32 = mybir.dt.float32
F32R = mybir.dt.float32r

# Full-size problem constants (hardcoded; the grading harness calls
# kernel(**inputs) with exactly these shapes).
B, E, N, D, H = 2, 8, 2048, 1024, 4096
N_CORES = 8


def _r(ap):
    """Bitcast an fp32 AP to float32r so the PE runs at 1 cycle/row."""
    return ap.bitcast(F32R)


def emit_expert_ffn(tc, x, w1, b1, w2, b2, y, hT_st, T, D_, H_, TCH=512, DCH=512,
                    use_b2=False):
    """Emit one expert's FFN. x:[T,D] w1:[D,H] b1:[H] w2:[H,D] b2:[D] y:[T,D].

    hT_st: [H, T] DRAM staging tile for the transposed activation.
    TCH: token chunk (matmul moving free dim) for GEMM1.
    DCH: output-dim chunk for GEMM2 (one PSUM bank).
    """
    nc = tc.nc
    NT = T // P          # token subtiles
    ND = D_ // P         # contraction tiles for GEMM1
    NH = H_ // P         # h tiles
    NCH = T // TCH       # token chunks
    G = min(4, NCH)      # chunks per PSUM group (4 banks)
    NG = NCH // G
    NDC = D_ // DCH      # output chunks for GEMM2

    assert T % P == 0 and D_ % P == 0 and H_ % P == 0
    assert T % TCH == 0 and NCH % G == 0 and D_ % DCH == 0

    hT_r = hT_st.rearrange("(ht p) t -> p ht t", p=P)

    with (
        tc.tile_pool(name="const", bufs=1) as const_pool,
    ):
        ident = const_pool.tile([P, P], F32)
        make_identity(nc, ident)
        b1_sb = const_pool.tile([P, NH], F32)
        nc.sync.dma_start(b1_sb[:], b1.rearrange("(ht p) -> p ht", p=P))
        if use_b2:
            b2_sb = const_pool.tile([P, D_], F32)
            # broadcast b2 across all 128 partitions
            nc.sync.dma_start(b2_sb[:], b2.unsqueeze(0).broadcast_to([P, D_]))

        # ---------------- Phase 1: xT, then hT = gelu(w1.T @ xT + b1) ------
        with (
            tc.tile_pool(name="xT_pool", bufs=1) as xT_pool,
            tc.tile_pool(name="xraw_pool", bufs=3) as xraw_pool,
            tc.tile_pool(name="w1_pool", bufs=6) as w1_pool,
            tc.tile_pool(name="hout_pool", bufs=2) as hout_pool,
            tc.tile_pool(name="ptp_pool", bufs=2, space="PSUM") as ptp_pool,
            tc.tile_pool(name="ph_pool", bufs=1, space="PSUM") as ph_pool,
        ):
            xT = xT_pool.tile([P, ND, T], F32R)
            for tsub in range(NT):
                x_raw = xraw_pool.tile([P, D_], F32, name="x_raw")
                nc.sync.dma_start(x_raw[:], x[tsub * P:(tsub + 1) * P, :])
                for dt in range(ND):
                    ptp = ptp_pool.tile([P, P], F32, name="ptp")
                    nc.tensor.transpose(ptp[:], x_raw[:, dt * P:(dt + 1) * P],
                                        ident[:])
                    nc.vector.tensor_copy(xT[:, dt, tsub * P:(tsub + 1) * P],
                                          ptp[:])

            for ht in range(NH):
                for g in range(NG):
                    psum_h = ph_pool.tile([P, G * TCH], F32, name="psum_h")
                    for dt in range(ND):
                        w1_t = w1_pool.tile([P, P], F32R, name="w1_t")
                        nc.sync.dma_start(
                            w1_t[:],
                            w1[dt * P:(dt + 1) * P, ht * P:(ht + 1) * P])
                        for i in range(G):
                            tc0 = (g * G + i) * TCH
                            nc.tensor.matmul(
                                psum_h[:, i * TCH:(i + 1) * TCH],
                                w1_t[:],
                                xT[:, dt, tc0:tc0 + TCH],
                                start=(dt == 0), stop=(dt == ND - 1))
                    hT_out = hout_pool.tile([P, G * TCH], F32R, name="hT_out")
                    for i in range(G):
                        # gelu(psum + b1[ht block]) straight out of PSUM
                        nc.scalar.activation(
                            hT_out[:, i * TCH:(i + 1) * TCH],
                            psum_h[:, i * TCH:(i + 1) * TCH],
                            mybir.ActivationFunctionType.Gelu_apprx_tanh,
                            bias=b1_sb[:, ht:ht + 1], scale=1.0)
                    nc.sync.dma_start(
                        hT_st[ht * P:(ht + 1) * P,
                              g * G * TCH:(g + 1) * G * TCH],
                        hT_out[:])

        # ---------------- Phase 2: y = hT.T @ w2 (+ b2) --------------------
        with (
            tc.tile_pool(name="w2_pool", bufs=1) as w2_pool,
            tc.tile_pool(name="hTin_pool", bufs=2) as hTin_pool,
            tc.tile_pool(name="out_pool", bufs=3) as out_pool,
            tc.tile_pool(name="po_pool", bufs=2, space="PSUM") as po_pool,
        ):
            w2_sb = w2_pool.tile([P, NH, D_], F32R)
            w2_r = w2.rearrange("(ht p) d -> p ht d", p=P)
            for ht in range(NH):
                nc.sync.dma_start(w2_sb[:, ht, :], w2_r[:, ht, :])

            for tt in range(NT):
                hT_in = hTin_pool.tile([P, NH, P], F32R, name="hT_in")
                nc.sync.dma_start(hT_in[:], hT_r[:, :, tt * P:(tt + 1) * P])
                psum_o = po_pool.tile([P, D_], F32, name="psum_o")
                for ht in range(NH):
                    for dc in range(NDC):
                        nc.tensor.matmul(
                            psum_o[:, dc * DCH:(dc + 1) * DCH],
                            hT_in[:, ht, :],
                            w2_sb[:, ht, dc * DCH:(dc + 1) * DCH],
                            start=(ht == 0), stop=(ht == NH - 1))
                out_sb = out_pool.tile([P, D_], F32, name="out_sb")
                if use_b2:
                    nc.vector.tensor_add(out_sb[:], psum_o[:], b2_sb[:])
                else:
                    nc.scalar.copy(out_sb[:], psum_o[:])
                nc.sync.dma_start(y[tt * P:(tt + 1) * P, :], out_sb[:])


def build_module(T, D_, H_, TCH=512, DCH=512, use_b2=False):
    nc = bacc.Bacc(None, target_bir_lowering=False)
    x = nc.dram_tensor("x", [T, D_], F32, kind="ExternalInput")
    w1 = nc.dram_tensor("w1", [D_, H_], F32R, kind="ExternalInput")
    b1 = nc.dram_tensor("b1", [H_], F32, kind="ExternalInput")
    w2 = nc.dram_tensor("w2", [H_, D_], F32R, kind="ExternalInput")
    if use_b2:
        b2 = nc.dram_tensor("b2", [D_], F32, kind="ExternalInput")
    else:
        b2 = None
    y = nc.dram_tensor("y", [T, D_], F32, kind="ExternalOutput")

    with tile.TileContext(nc) as tc:
        with tc.tile_pool(name="dram_st", bufs=1, space="DRAM") as dram_pool:
            hT_st = dram_pool.tile([H_, T], F32R)
            emit_expert_ffn(tc, x[:], w1[:], b1[:], w2[:],
                            b2[:] if use_b2 else None, y[:], hT_st,
                            T, D_, H_, TCH=TCH, DCH=DCH, use_b2=use_b2)
    nc.compile()
    return nc


_module_cache = {}


def _get_module(key):
    if key not in _module_cache:
        T, D_, H_, use_b2 = key
        _module_cache[key] = build_module(T, D_, H_, use_b2=use_b2)
    return _module_cache[key]


def run_moe(x, w1, b1, w2, b2, trace=False):
    """x:(B,E,N,D) w1:(E,D,H) b1:(E,H) w2:(E,H,D) b2:(E,D) -> (B,E,N,D)."""
    Bx, Ex, Nx, Dx = x.shape
    Hx = w1.shape[2]
    T = Bx * Nx
    use_b2 = bool(np.any(b2))
    nc = _get_module((T, Dx, Hx, use_b2))

    in_maps = []
    for e in range(Ex):
        m = {
            "x": np.ascontiguousarray(x[:, e]).reshape(T, Dx),
            "w1": np.ascontiguousarray(w1[e]),
            "b1": np.ascontiguousarray(b1[e]),
            "w2": np.ascontiguousarray(w2[e]),
        }
        if use_b2:
            m["b2"] = np.ascontiguousarray(b2[e])
        in_maps.append(m)

    br = run_bass_kernel_spmd(nc, in_maps, core_ids=list(range(Ex)),
                              trace=trace)
    ys = np.stack([br.results[e]["y"] for e in range(Ex)], axis=0)  # [E,T,D]
    out = ys.reshape(Ex, Bx, Nx, Dx).reshape(Bx, Ex, Nx, Dx)
    return (out, br) if trace else (out, None)


def kernel(x, w1, b1, w2, b2):
    out, _ = run_moe(np.asarray(x), np.asarray(w1), np.asarray(b1),
                     np.asarray(w2), np.asarray(b2))
    return out
